# revision 26
# baseline (speedup 1.0000x reference)
"""MoE-routed group-norm kernel for Trainium2 (Bass/Tile), 8-core SPMD.

Problem (hardcoded shapes):
  x: [64, 512, 32, 32] f32
  experts_weight/bias: [8, 512], shared_weight/bias: [512]
  router_w: [8, 512], router_b: [8]

  flat = x.mean((2,3)); logits = flat @ router_w.T + router_b
  prob = softmax(logits); top-2 -> coeff = vals / sum(vals)
  fused_w = sum_k coeff_k * experts_weight[idx_k] + shared_weight (bias likewise)
  group-norm over G=32 groups of 16 channels, then y = x_norm * fused_w + fused_b

Strategy: data-parallel over batch, 8 samples per core, processed in PAIRS
(one 4 MiB load + one 4 MiB store per pair -> 8 big SWDGE DMAs total).
Channels live on partitions ([512,1024] = 4 chunks of [128,1024]).

Per-channel sums: S1 via DVE reduce_sum, S2 via ACT Square+accum_out (the
square's bulk output lands in a scratch tile). Every cross-partition step
(logits matvec, group-of-16 sums, group->channel broadcast, expert mixing,
[2,8]->[8,2] coeff transpose) is a tiny PE matmul against constant masks.

Routing math runs pair-batched in a [2, E] layout (pair on partitions) with a
single Exp per pair: top-1 exp is exactly 1.0 and the softmax denominator
cancels in coeff = vals/sum(vals), so is_lt/is_ge masking replaces index math.

rstd = 1/sqrt(var+eps) is computed on DVE with the bit-trick rsqrt seed plus
three Newton iterations (fp32-exact to ~1e-9 relative) — keeping ACT's
activation-table fixed at exp_and_others (exp/square/identity), i.e. ONE
ACT table load for the whole kernel instead of one per exp/sqrt alternation.

Output pass y = x*A + B (A = fused_w*rstd, B = fused_b - mean*A) as one
fused per-partition-scalar op per chunk, split 2 chunks DVE / 2 chunks ACT.
"""

import numpy as np

import concourse.bacc as bacc
import concourse.bass as bass
import concourse.tile as tile
from concourse import mybir
from concourse.bass_utils import run_bass_kernel_spmd

F32 = mybir.dt.float32
I32 = mybir.dt.int32
ALU = mybir.AluOpType
ACTF = mybir.ActivationFunctionType
AXX = mybir.AxisListType.X

P = 128            # SBUF partitions
B, C, HWD = 64, 512, 1024
E, G = 8, 32
EPS = 1e-5
NCORES = 8
BPC = B // NCORES  # samples per core
NCH = C // P       # 4 channel chunks per sample
CPG = C // G       # 16 channels per group
PAIR = 2
NPAIR = BPC // PAIR
RSQRT_MAGIC = 0x5F3759DF

# cA layout [128, 56]:
#   0:32  routerT   (routerT[p, 8j+e] = router_w[e, 128j+p] / 1024)
#   32:40 gmask     (1 if p//16 == g)
#   40:48 sw2       ((j, s): shared_weight[128j+p], replicated over s=0,1)
#   48:56 sb2       (same for shared_bias)
CA_W = 56
# cB layout [8, 1164]:
#   0:128    bmask  (1 if p//16 == g)
#   128:640  experts_weight
#   640:1152 experts_bias
#   1152:1160 rb2   (rows 0:2 = router_b, replicated)
#   1160:1162 ident2 (rows 0:2 = 2x2 identity)
CB_W = 1162


def build(n_b: int = BPC) -> bass.Bass:
    assert n_b % PAIR == 0
    npair = n_b // PAIR
    # Bacc (not plain Bass): finalize() runs move_matmul_waits_to_ldweights +
    # generate_event_semaphores, splitting multi-sem waits to satisfy the
    # one-wait-per-instruction hardware constraint.
    nc = bacc.Bacc()
    x_d = nc.declare_dram_parameter("x", [n_b, C, HWD], F32, isOutput=False)
    ca_d = nc.declare_dram_parameter("ca", [P, CA_W], F32, isOutput=False)
    cb_d = nc.declare_dram_parameter("cb", [E, CB_W], F32, isOutput=False)
    y_d = nc.declare_dram_parameter("y", [n_b, C, HWD], F32, isOutput=True)

    with tile.TileContext(nc) as tc:
        with (
            tc.tile_pool(name="consts", bufs=1) as consts,
            tc.tile_pool(name="xp", bufs=2) as xp,
            tc.tile_pool(name="yp", bufs=2) as yp,
            tc.tile_pool(name="scr", bufs=4) as scrp,
            tc.tile_pool(name="statp", bufs=3) as statp,
            tc.tile_pool(name="tinyp", bufs=3) as tinyp,
            tc.tile_pool(name="ps_static", bufs=1, space="PSUM") as pstat,
        ):
            # consts staged through a DVE copy: PE inputs need single-engine
            # (DVE) provenance so matmuls carry at most one sync wait.
            ca_st = consts.tile([P, CA_W], F32)
            nc.sync.dma_start(out=ca_st, in_=ca_d[:, :])
            cb_st = consts.tile([E, CB_W], F32)
            nc.sync.dma_start(out=cb_st, in_=cb_d[:, :])
            ca = consts.tile([P, CA_W], F32)
            nc.vector.tensor_copy(ca, ca_st)
            cb = consts.tile([E, CB_W], F32)
            nc.vector.tensor_copy(cb, cb_st)
            zeros128 = consts.tile([P, 1], F32)
            nc.vector.memset(zeros128, 0.0)
            # rsqrt constants, written through int32 views
            magic8 = consts.tile([E, PAIR * NCH], F32)
            nc.vector.memset(magic8[:, :].bitcast(I32), RSQRT_MAGIC)
            one8i = consts.tile([E, PAIR * NCH], F32)
            nc.vector.memset(one8i[:, :].bitcast(I32), 1)

            gmask = ca[:, 32:40]
            sw2 = ca[:, 40:48].rearrange("p (j s) -> p j s", s=PAIR)
            sb2 = ca[:, 48:56].rearrange("p (j s) -> p j s", s=PAIR)
            bmask = cb[:, 0:P]
            rb2 = cb[0:PAIR, 1152:1160]
            ident2 = cb[0:PAIR, 1160:1162]

            # static per-pair PSUM regions (never reused -> no PSUM WAW deps)
            ps_sm = pstat.tile([E, 26 * npair], F32, tag="sm")
            ps_fu = pstat.tile([P, npair, 8, PAIR], F32, tag="fu")
            ps_bc = pstat.tile([P, npair, PAIR, NCH, 2], F32, tag="bc")
            # ACT-written erow gets static regions (no ACT WAW waits)
            erow_all = consts.tile([PAIR, npair, E], F32)

            for ip in range(npair):
                x_t2 = xp.tile([P, PAIR, NCH, HWD], F32)
                nc.gpsimd.dma_start(
                    out=x_t2,
                    in_=x_d[ip * PAIR : (ip + 1) * PAIR].rearrange(
                        "b (t p) f -> p b t f", p=P
                    ),
                )
                y_t2 = yp.tile([P, PAIR, NCH, HWD], F32)
                # absorb the y-slot WAR (old store DMA) into one DVE touch
                nc.vector.memset(y_t2[0:1, 0, 0, 0:1], 0.0)

                # per-channel sums: S1 (DVE reduce), S2 (ACT square+accum)
                s1 = statp.tile([P, PAIR, NCH], F32, tag="s1")
                s2 = statp.tile([P, PAIR, NCH], F32, tag="s2")
                for bb in range(PAIR):
                    for j in range(NCH):
                        nc.vector.reduce_sum(
                            s1[:, bb, j : j + 1], x_t2[:, bb, j, :], axis=AXX
                        )
                        sq = scrp.tile([P, HWD], F32, tag="sq")
                        nc.scalar.activation(
                            sq,
                            x_t2[:, bb, j, :],
                            ACTF.Square,
                            bias=zeros128,
                            scale=1.0,
                            accum_out=s2[:, bb, j : j + 1],
                        )

                o = 26 * ip
                gs1_ps = ps_sm[:, o : o + 8]          # group sums of S1 (bb,j)
                gs2_ps = ps_sm[:, o + 8 : o + 16]     # group sums of S2 (bb,j)
                lg_ps = ps_sm[0:PAIR, o + 16 : o + 24]  # logits [2, 8]
                ct_ps = ps_sm[:, o + 24 : o + 26]     # coeff^T [8, 2]

                # logits[s, e] = sum_c S1[c, s]/1024 * router_w[e, c]
                for j in range(NCH):
                    nc.tensor.matmul(
                        lg_ps,
                        s1[:, :, j],
                        ca[:, j * 8 : (j + 1) * 8],
                        start=(j == 0),
                        stop=(j == NCH - 1),
                    )
                nc.tensor.matmul(gs1_ps, gmask, s1[:, :, :])
                nc.tensor.matmul(gs2_ps, gmask, s2[:, :, :])

                # routing, pair-batched in [2, E] partition layout
                lrow = tinyp.tile([PAIR, E], F32, tag="lrow")
                nc.vector.tensor_tensor(lrow, lg_ps, rb2, ALU.add)
                nmax = tinyp.tile([PAIR, 1], F32, tag="nmax")
                nc.vector.reduce_max(nmax, lrow, axis=AXX, negate=True)
                erow = erow_all[:, ip, :]
                nc.scalar.activation(erow, lrow, ACTF.Exp, bias=nmax, scale=1.0)
                qrow = tinyp.tile([PAIR, E], F32, tag="qrow")
                nc.vector.scalar_tensor_tensor(
                    qrow, erow, 1.0, erow, op0=ALU.is_lt, op1=ALU.mult
                )
                m2 = tinyp.tile([PAIR, 1], F32, tag="m2")
                nc.vector.reduce_max(m2, qrow, axis=AXX)
                gate = tinyp.tile([PAIR, E], F32, tag="gate")
                nc.vector.scalar_tensor_tensor(
                    gate, erow, m2[:, 0:1], erow, op0=ALU.is_ge, op1=ALU.mult
                )
                den = tinyp.tile([PAIR, 1], F32, tag="den")
                nc.vector.tensor_scalar_add(den, m2, 1.0)
                rden = tinyp.tile([PAIR, 1], F32, tag="rden")
                nc.vector.reciprocal(rden, den)
                crow = tinyp.tile([PAIR, E], F32, tag="crow")
                nc.vector.tensor_scalar_mul(crow, gate, rden[:, 0:1])
                # transpose coeff [2,8] -> [8,2]: lhsT=crow, rhs=I2
                nc.tensor.matmul(ct_ps, crow, ident2)
                cT = tinyp.tile([E, PAIR], F32, tag="cT")
                nc.vector.tensor_copy(cT, ct_ps)

                # group stats -> mean, rstd in mr [8, (bb, j), 2]
                mr = statp.tile([E, PAIR, NCH, 2], F32, tag="mr")
                mean8 = mr[:, :, :, 0].rearrange("g b j -> g (b j)")
                nc.vector.tensor_scalar_mul(mean8, gs1_ps, 1.0 / (CPG * HWD))
                ex2 = tinyp.tile([E, PAIR * NCH], F32, tag="ex2")
                nc.vector.tensor_scalar_mul(ex2, gs2_ps, 1.0 / (CPG * HWD))
                mg2 = tinyp.tile([E, PAIR * NCH], F32, tag="mg2")
                nc.vector.tensor_tensor(mg2, mean8, mean8, ALU.mult)
                # v = (ex2 + eps) - mean^2
                v = tinyp.tile([E, PAIR * NCH], F32, tag="v")
                nc.vector.scalar_tensor_tensor(
                    v, ex2, EPS, mg2, op0=ALU.add, op1=ALU.subtract
                )
                # rstd = rsqrt(v): bit-trick seed + 3 Newton iterations (DVE
                # only, keeps ACT's table pinned to exp_and_others)
                yr = tinyp.tile([E, PAIR * NCH], F32, tag="yr")
                nc.vector.tensor_tensor(
                    yr[:, :].bitcast(I32),
                    v[:, :].bitcast(I32),
                    one8i[:, :].bitcast(I32),
                    ALU.arith_shift_right,
                )
                nc.vector.tensor_tensor(
                    yr[:, :].bitcast(I32),
                    magic8[:, :].bitcast(I32),
                    yr[:, :].bitcast(I32),
                    ALU.subtract,
                )
                t_a = tinyp.tile([E, PAIR * NCH], F32, tag="t_a")
                t_b = tinyp.tile([E, PAIR * NCH], F32, tag="t_b")
                for _ in range(3):
                    nc.vector.tensor_tensor(t_a, yr, yr, ALU.mult)
                    nc.vector.tensor_tensor(t_b, t_a, v, ALU.mult)
                    nc.vector.tensor_scalar(
                        t_a, t_b, -0.5, 1.5, op0=ALU.mult, op1=ALU.add
                    )
                    nc.vector.tensor_tensor(yr, yr, t_a, ALU.mult)
                nc.vector.tensor_copy(
                    mr[:, :, :, 1].rearrange("g b j -> g (b j)"), yr
                )

                # broadcast group stats to channels and mix expert tables
                bc = ps_bc[:, ip, :, :, :]
                nc.tensor.matmul(bc, bmask, mr[:, :, :, :])
                fu = ps_fu[:, ip, :, :]
                for j in range(NCH):
                    nc.tensor.matmul(
                        fu[:, j, :], cb[:, P + j * P : P + (j + 1) * P], cT
                    )
                    nc.tensor.matmul(
                        fu[:, NCH + j, :], cb[:, 640 + j * P : 640 + (j + 1) * P], cT
                    )

                # A = (fused_w + shared_w) * rstd ; B = (fused_b+shared_b) - mean*A
                bc_mean = bc[:, :, :, 0].rearrange("p b j -> p j b")
                bc_rstd = bc[:, :, :, 1].rearrange("p b j -> p j b")
                t1 = tinyp.tile([P, NCH, PAIR], F32, tag="t1")
                nc.vector.tensor_tensor(t1, fu[:, 0:NCH, :], sw2, ALU.add)
                At = tinyp.tile([P, NCH, PAIR], F32, tag="At")
                nc.vector.tensor_tensor(At, t1, bc_rstd, ALU.mult)
                t2 = tinyp.tile([P, NCH, PAIR], F32, tag="t2")
                nc.vector.tensor_tensor(t2, fu[:, NCH : 2 * NCH, :], sb2, ALU.add)
                t3 = tinyp.tile([P, NCH, PAIR], F32, tag="t3")
                nc.vector.tensor_tensor(t3, bc_mean, At, ALU.mult)
                Bt = tinyp.tile([P, NCH, PAIR], F32, tag="Bt")
                nc.vector.tensor_tensor(Bt, t2, t3, ALU.subtract)

                # pass2: y = x*A + B, split 2 chunks DVE / 2 chunks ACT
                for bb in range(PAIR):
                    for j in range(NCH):
                        if j < 2:
                            nc.vector.tensor_scalar(
                                y_t2[:, bb, j, :],
                                x_t2[:, bb, j, :],
                                At[:, j, bb : bb + 1],
                                Bt[:, j, bb : bb + 1],
                                op0=ALU.mult,
                                op1=ALU.add,
                            )
                        else:
                            nc.scalar.activation(
                                y_t2[:, bb, j, :],
                                x_t2[:, bb, j, :],
                                ACTF.Identity,
                                bias=Bt[:, j, bb : bb + 1],
                                scale=At[:, j, bb : bb + 1],
                            )

                nc.gpsimd.dma_start(
                    out=y_d[ip * PAIR : (ip + 1) * PAIR].rearrange(
                        "b (t p) f -> p b t f", p=P
                    ),
                    in_=y_t2,
                )
    nc.finalize()
    return nc


def pack_consts(
    experts_weight, experts_bias, shared_weight, shared_bias, router_w, router_b
):
    ca = np.zeros((P, CA_W), np.float32)
    ca[:, 0:32] = (
        (np.ascontiguousarray(router_w.T) / HWD)
        .reshape(NCH, P, E)
        .transpose(1, 0, 2)
        .reshape(P, 32)
    )
    pidx = np.arange(P)
    ca[:, 32:40] = (pidx[:, None] // CPG == np.arange(8)[None, :]).astype(np.float32)
    sw = shared_weight.reshape(NCH, P).T  # [128, 4]
    sb = shared_bias.reshape(NCH, P).T
    ca[:, 40:48] = np.repeat(sw, PAIR, axis=1)
    ca[:, 48:56] = np.repeat(sb, PAIR, axis=1)
    cb = np.zeros((E, CB_W), np.float32)
    cb[:, 0:P] = (np.arange(E)[:, None] == pidx[None, :] // CPG).astype(np.float32)
    cb[:, P : P + C] = experts_weight
    cb[:, P + C : P + 2 * C] = experts_bias
    cb[0:PAIR, 1152:1160] = router_b[None, :]
    cb[0:PAIR, 1160:1162] = np.eye(PAIR, dtype=np.float32)
    return ca, cb


_NC_CACHE: dict[int, bass.Bass] = {}


def _get_nc(n_b: int) -> bass.Bass:
    if n_b not in _NC_CACHE:
        _NC_CACHE[n_b] = build(n_b)
    return _NC_CACHE[n_b]


def run(
    x,
    experts_weight,
    experts_bias,
    shared_weight,
    shared_bias,
    router_w,
    router_b,
    trace: bool = False,
    tmpdir=None,
):
    x = np.ascontiguousarray(np.asarray(x, np.float32)).reshape(B, C, HWD)
    ca, cb = pack_consts(
        np.asarray(experts_weight, np.float32),
        np.asarray(experts_bias, np.float32),
        np.asarray(shared_weight, np.float32),
        np.asarray(shared_bias, np.float32),
        np.asarray(router_w, np.float32),
        np.asarray(router_b, np.float32),
    )
    nc = _get_nc(BPC)
    in_maps = [
        {"x": x[i * BPC : (i + 1) * BPC], "ca": ca, "cb": cb} for i in range(NCORES)
    ]
    res = run_bass_kernel_spmd(
        nc, in_maps, list(range(NCORES)), trace=trace, tmpdir=tmpdir
    )
    y = np.concatenate([res.results[i]["y"] for i in range(NCORES)], axis=0)
    return y.reshape(B, C, 32, 32), res


def kernel(**inputs) -> np.ndarray:
    y, _ = run(**inputs)
    return y


# revision 31
# speedup vs baseline: 1.3732x; 1.3732x over previous
"""MoE-routed group-norm kernel for Trainium2 (Bass/Tile), 8-core SPMD.

Problem (hardcoded shapes):
  x: [64, 512, 32, 32] f32
  experts_weight/bias: [8, 512], shared_weight/bias: [512]
  router_w: [8, 512], router_b: [8]

  flat = x.mean((2,3)); logits = flat @ router_w.T + router_b
  prob = softmax(logits); top-2 -> coeff = vals / sum(vals)
  fused_w = sum_k coeff_k * experts_weight[idx_k] + shared_weight (bias likewise)
  group-norm over G=32 groups of 16 channels, then y = x_norm * fused_w + fused_b

Strategy: data-parallel over batch, 8 samples per core. Channels on
partitions ([512,1024] = 4 chunks of [128,1024] per sample). The three
full-tensor streaming passes are spread across three engines so the kernel
stays DMA-bound:
  S1 (per-channel sums)    -> GpSimd reduce_sum
  S2 (per-channel sum x^2) -> ACT Square + accum_out (bulk out to scratch)
  pass2 (y = x*A + B)      -> DVE tensor_scalar (2x fp32 mode)
DMAs are HWDGE: loads on the SP ring (2 x 1 MiB per sample for early compute
start), stores on the ACT ring (2 MiB per sample). Bacc's finalize() splits
multi-sem waits (one-wait-per-instruction hardware constraint), which is what
makes HWDGE DMAs legal here.

All cross-partition steps (logits matvec, group-of-16 sums, group->channel
broadcast, expert mixing, [2,8]->[8,2] coeff transpose) are tiny PE matmuls
against constant masks, batched per PAIR of samples. Routing runs in a [2, E]
layout (pair on partitions, one Exp per pair): top-1 exp is exactly 1.0 and
the softmax denominator cancels in coeff = vals/sum(vals), so is_lt/is_ge
masking replaces any index math. rstd = 1/sqrt(var+eps) uses the bit-trick
seed + 3 Newton steps on DVE, keeping ACT's table pinned to exp_and_others
(exp/square/identity -> a single ACT table load for the whole kernel).
PSUM and ACT-written tiles use static per-pair regions (no slot reuse, no
cross-iteration WAW completion waits on PE/ACT).
"""

import numpy as np

import concourse.bacc as bacc
import concourse.bass as bass
import concourse.tile as tile
from concourse import mybir
from concourse.bass_utils import run_bass_kernel_spmd

F32 = mybir.dt.float32
I32 = mybir.dt.int32
ALU = mybir.AluOpType
ACTF = mybir.ActivationFunctionType
AXX = mybir.AxisListType.X

P = 128            # SBUF partitions
B, C, HWD = 64, 512, 1024
E, G = 8, 32
EPS = 1e-5
NCORES = 8
BPC = B // NCORES  # samples per core
NCH = C // P       # 4 channel chunks per sample
CPG = C // G       # 16 channels per group
PAIR = 2
RSQRT_MAGIC = 0x5F3759DF

# cA layout [128, 56]:
#   0:32  routerT   (routerT[p, 8j+e] = router_w[e, 128j+p] / 1024)
#   32:40 gmask     (1 if p//16 == g)
#   40:48 sw2       ((j, s): shared_weight[128j+p], replicated over s=0,1)
#   48:56 sb2      (same for shared_bias)
CA_W = 56
# cB layout [8, 1162]:
#   0:128 bmask | 128:640 ew | 640:1152 eb | 1152:1160 rb2 | 1160:1162 ident2
CB_W = 1162


def build(n_b: int = BPC) -> bass.Bass:
    assert n_b % PAIR == 0
    npair = n_b // PAIR
    nc = bacc.Bacc()
    x_d = nc.declare_dram_parameter("x", [n_b, C, HWD], F32, isOutput=False)
    ca_d = nc.declare_dram_parameter("ca", [P, CA_W], F32, isOutput=False)
    cb_d = nc.declare_dram_parameter("cb", [E, CB_W], F32, isOutput=False)
    y_d = nc.declare_dram_parameter("y", [n_b, C, HWD], F32, isOutput=True)

    with tile.TileContext(nc) as tc:
        with (
            tc.tile_pool(name="consts", bufs=1) as consts,
            tc.tile_pool(name="xp", bufs=5) as xp,
            tc.tile_pool(name="yp", bufs=3) as yp,
            tc.tile_pool(name="scr", bufs=4) as scrp,
            tc.tile_pool(name="statp", bufs=3) as statp,
            tc.tile_pool(name="tinyp", bufs=3) as tinyp,
            tc.tile_pool(name="ps_static", bufs=1, space="PSUM") as pstat,
        ):
            # consts staged through a DVE copy so PE inputs have DVE provenance
            ca_st = consts.tile([P, CA_W], F32)
            nc.sync.dma_start(out=ca_st, in_=ca_d[:, :])
            cb_st = consts.tile([E, CB_W], F32)
            nc.sync.dma_start(out=cb_st, in_=cb_d[:, :])
            ca = consts.tile([P, CA_W], F32)
            nc.vector.tensor_copy(ca, ca_st)
            cb = consts.tile([E, CB_W], F32)
            nc.vector.tensor_copy(cb, cb_st)
            zeros128 = consts.tile([P, 1], F32)
            nc.vector.memset(zeros128, 0.0)
            magic8 = consts.tile([E, PAIR * NCH], F32)
            nc.vector.memset(magic8[:, :].bitcast(I32), RSQRT_MAGIC)
            one8i = consts.tile([E, PAIR * NCH], F32)
            nc.vector.memset(one8i[:, :].bitcast(I32), 1)

            gmask = ca[:, 32:40]
            sw2 = ca[:, 40:48].rearrange("p (j s) -> p j s", s=PAIR)
            sb2 = ca[:, 48:56].rearrange("p (j s) -> p j s", s=PAIR)
            bmask = cb[:, 0:P]
            rb2 = cb[0:PAIR, 1152:1160]
            ident2 = cb[0:PAIR, 1160:1162]

            # static per-pair PSUM regions (never reused -> no PSUM WAW deps)
            ps_sm = pstat.tile([E, 26 * npair], F32, tag="sm")
            ps_fu = pstat.tile([P, npair, 8, PAIR], F32, tag="fu")
            ps_bc = pstat.tile([P, npair, PAIR, NCH, 2], F32, tag="bc")
            erow_all = consts.tile([PAIR, npair, E], F32)

            for ip in range(npair):
                xts = []
                s1 = statp.tile([P, PAIR, NCH], F32, tag="s1")
                s2 = statp.tile([P, PAIR, NCH], F32, tag="s2")
                for bb in range(PAIR):
                    b = ip * PAIR + bb
                    x_t = xp.tile([P, NCH, HWD], F32, tag="x")
                    xts.append(x_t)
                    xv = x_d[b].rearrange("(t p) f -> p t f", p=P)
                    # two 1 MiB loads so stats start after half the sample
                    nc.sync.dma_start(out=x_t[:, 0:2, :], in_=xv[:, 0:2, :])
                    nc.sync.dma_start(out=x_t[:, 2:4, :], in_=xv[:, 2:4, :])
                    for j in range(NCH):
                        nc.vector.reduce_sum(
                            s1[:, bb, j : j + 1], x_t[:, j, :], axis=AXX
                        )
                        sq = scrp.tile([P, HWD], F32, tag="sq")
                        nc.scalar.activation(
                            sq,
                            x_t[:, j, :],
                            ACTF.Square,
                            bias=zeros128,
                            scale=1.0,
                            accum_out=s2[:, bb, j : j + 1],
                        )

                o = 26 * ip
                gs1_ps = ps_sm[:, o : o + 8]          # group sums of S1 (bb,j)
                gs2_ps = ps_sm[:, o + 8 : o + 16]     # group sums of S2 (bb,j)
                lg_ps = ps_sm[0:PAIR, o + 16 : o + 24]  # logits [2, 8]
                ct_ps = ps_sm[:, o + 24 : o + 26]     # coeff^T [8, 2]

                # logits[s, e] = sum_c S1[c, s]/1024 * router_w[e, c]
                for j in range(NCH):
                    nc.tensor.matmul(
                        lg_ps,
                        s1[:, :, j],
                        ca[:, j * 8 : (j + 1) * 8],
                        start=(j == 0),
                        stop=(j == NCH - 1),
                    )
                nc.tensor.matmul(gs1_ps, gmask, s1[:, :, :])
                nc.tensor.matmul(gs2_ps, gmask, s2[:, :, :])

                # routing, pair-batched in [2, E] partition layout
                lrow = tinyp.tile([PAIR, E], F32, tag="lrow")
                nc.vector.tensor_tensor(lrow, lg_ps, rb2, ALU.add)
                nmax = tinyp.tile([PAIR, 1], F32, tag="nmax")
                nc.vector.reduce_max(nmax, lrow, axis=AXX, negate=True)
                erow = erow_all[:, ip, :]
                nc.scalar.activation(erow, lrow, ACTF.Exp, bias=nmax, scale=1.0)
                qrow = tinyp.tile([PAIR, E], F32, tag="qrow")
                nc.vector.scalar_tensor_tensor(
                    qrow, erow, 1.0, erow, op0=ALU.is_lt, op1=ALU.mult
                )
                m2 = tinyp.tile([PAIR, 1], F32, tag="m2")
                nc.vector.reduce_max(m2, qrow, axis=AXX)
                gate = tinyp.tile([PAIR, E], F32, tag="gate")
                nc.vector.scalar_tensor_tensor(
                    gate, erow, m2[:, 0:1], erow, op0=ALU.is_ge, op1=ALU.mult
                )
                den = tinyp.tile([PAIR, 1], F32, tag="den")
                nc.vector.tensor_scalar_add(den, m2, 1.0)
                rden = tinyp.tile([PAIR, 1], F32, tag="rden")
                nc.vector.reciprocal(rden, den)
                crow = tinyp.tile([PAIR, E], F32, tag="crow")
                nc.vector.tensor_scalar_mul(crow, gate, rden[:, 0:1])
                nc.tensor.matmul(ct_ps, crow, ident2)
                cT = tinyp.tile([E, PAIR], F32, tag="cT")
                nc.vector.tensor_copy(cT, ct_ps)

                # group stats -> mean, rstd in mr [8, (bb, j), 2]
                mr = statp.tile([E, PAIR, NCH, 2], F32, tag="mr")
                mean8 = mr[:, :, :, 0].rearrange("g b j -> g (b j)")
                nc.vector.tensor_scalar_mul(mean8, gs1_ps, 1.0 / (CPG * HWD))
                ex2 = tinyp.tile([E, PAIR * NCH], F32, tag="ex2")
                nc.vector.tensor_scalar_mul(ex2, gs2_ps, 1.0 / (CPG * HWD))
                mg2 = tinyp.tile([E, PAIR * NCH], F32, tag="mg2")
                nc.vector.tensor_tensor(mg2, mean8, mean8, ALU.mult)
                v = tinyp.tile([E, PAIR * NCH], F32, tag="v")
                nc.vector.scalar_tensor_tensor(
                    v, ex2, EPS, mg2, op0=ALU.add, op1=ALU.subtract
                )
                # rstd = rsqrt(v): bit-trick seed + 3 Newton iterations (DVE)
                yr = tinyp.tile([E, PAIR * NCH], F32, tag="yr")
                nc.vector.tensor_tensor(
                    yr[:, :].bitcast(I32),
                    v[:, :].bitcast(I32),
                    one8i[:, :].bitcast(I32),
                    ALU.arith_shift_right,
                )
                nc.vector.tensor_tensor(
                    yr[:, :].bitcast(I32),
                    magic8[:, :].bitcast(I32),
                    yr[:, :].bitcast(I32),
                    ALU.subtract,
                )
                t_a = tinyp.tile([E, PAIR * NCH], F32, tag="t_a")
                t_b = tinyp.tile([E, PAIR * NCH], F32, tag="t_b")
                for _ in range(3):
                    nc.vector.tensor_tensor(t_a, yr, yr, ALU.mult)
                    nc.vector.tensor_tensor(t_b, t_a, v, ALU.mult)
                    nc.vector.tensor_scalar(
                        t_a, t_b, -0.5, 1.5, op0=ALU.mult, op1=ALU.add
                    )
                    nc.vector.tensor_tensor(yr, yr, t_a, ALU.mult)
                nc.vector.tensor_copy(
                    mr[:, :, :, 1].rearrange("g b j -> g (b j)"), yr
                )

                # broadcast group stats to channels and mix expert tables
                bc = ps_bc[:, ip, :, :, :]
                nc.tensor.matmul(bc, bmask, mr[:, :, :, :])
                fu = ps_fu[:, ip, :, :]
                for j in range(NCH):
                    nc.tensor.matmul(
                        fu[:, j, :], cb[:, P + j * P : P + (j + 1) * P], cT
                    )
                    nc.tensor.matmul(
                        fu[:, NCH + j, :], cb[:, 640 + j * P : 640 + (j + 1) * P], cT
                    )

                # A = (fused_w + shared_w) * rstd ; B = (fused_b+shared_b) - mean*A
                bc_mean = bc[:, :, :, 0].rearrange("p b j -> p j b")
                bc_rstd = bc[:, :, :, 1].rearrange("p b j -> p j b")
                t1 = tinyp.tile([P, NCH, PAIR], F32, tag="t1")
                nc.vector.tensor_tensor(t1, fu[:, 0:NCH, :], sw2, ALU.add)
                At = tinyp.tile([P, NCH, PAIR], F32, tag="At")
                nc.vector.tensor_tensor(At, t1, bc_rstd, ALU.mult)
                t2 = tinyp.tile([P, NCH, PAIR], F32, tag="t2")
                nc.vector.tensor_tensor(t2, fu[:, NCH : 2 * NCH, :], sb2, ALU.add)
                t3 = tinyp.tile([P, NCH, PAIR], F32, tag="t3")
                nc.vector.tensor_tensor(t3, bc_mean, At, ALU.mult)
                Bt = tinyp.tile([P, NCH, PAIR], F32, tag="Bt")
                nc.vector.tensor_tensor(Bt, t2, t3, ALU.subtract)

                # pass2 on GpSimd (frees DVE/ACT), per-sample store on ACT ring
                for bb in range(PAIR):
                    b = ip * PAIR + bb
                    y_t = yp.tile([P, NCH, HWD], F32, tag="y")
                    for j in range(NCH):
                        if j < 2:
                            nc.vector.tensor_scalar(
                                y_t[:, j, :],
                                xts[bb][:, j, :],
                                At[:, j, bb : bb + 1],
                                Bt[:, j, bb : bb + 1],
                                op0=ALU.mult,
                                op1=ALU.add,
                            )
                        else:
                            nc.scalar.activation(
                                y_t[:, j, :],
                                xts[bb][:, j, :],
                                ACTF.Identity,
                                bias=Bt[:, j, bb : bb + 1],
                                scale=At[:, j, bb : bb + 1],
                            )
                    nc.scalar.dma_start(
                        out=y_d[b].rearrange("(t p) f -> p t f", p=P), in_=y_t
                    )
    nc.finalize()
    return nc


def pack_consts(
    experts_weight, experts_bias, shared_weight, shared_bias, router_w, router_b
):
    ca = np.zeros((P, CA_W), np.float32)
    ca[:, 0:32] = (
        (np.ascontiguousarray(router_w.T) / HWD)
        .reshape(NCH, P, E)
        .transpose(1, 0, 2)
        .reshape(P, 32)
    )
    pidx = np.arange(P)
    ca[:, 32:40] = (pidx[:, None] // CPG == np.arange(8)[None, :]).astype(np.float32)
    sw = shared_weight.reshape(NCH, P).T
    sb = shared_bias.reshape(NCH, P).T
    ca[:, 40:48] = np.repeat(sw, PAIR, axis=1)
    ca[:, 48:56] = np.repeat(sb, PAIR, axis=1)
    cb = np.zeros((E, CB_W), np.float32)
    cb[:, 0:P] = (np.arange(E)[:, None] == pidx[None, :] // CPG).astype(np.float32)
    cb[:, P : P + C] = experts_weight
    cb[:, P + C : P + 2 * C] = experts_bias
    cb[0:PAIR, 1152:1160] = router_b[None, :]
    cb[0:PAIR, 1160:1162] = np.eye(PAIR, dtype=np.float32)
    return ca, cb


_NC_CACHE: dict[int, bass.Bass] = {}


def _get_nc(n_b: int) -> bass.Bass:
    if n_b not in _NC_CACHE:
        _NC_CACHE[n_b] = build(n_b)
    return _NC_CACHE[n_b]


def run(
    x,
    experts_weight,
    experts_bias,
    shared_weight,
    shared_bias,
    router_w,
    router_b,
    trace: bool = False,
    tmpdir=None,
):
    x = np.ascontiguousarray(np.asarray(x, np.float32)).reshape(B, C, HWD)
    ca, cb = pack_consts(
        np.asarray(experts_weight, np.float32),
        np.asarray(experts_bias, np.float32),
        np.asarray(shared_weight, np.float32),
        np.asarray(shared_bias, np.float32),
        np.asarray(router_w, np.float32),
        np.asarray(router_b, np.float32),
    )
    nc = _get_nc(BPC)
    in_maps = [
        {"x": x[i * BPC : (i + 1) * BPC], "ca": ca, "cb": cb} for i in range(NCORES)
    ]
    res = run_bass_kernel_spmd(
        nc, in_maps, list(range(NCORES)), trace=trace, tmpdir=tmpdir
    )
    y = np.concatenate([res.results[i]["y"] for i in range(NCORES)], axis=0)
    return y.reshape(B, C, 32, 32), res


def kernel(**inputs) -> np.ndarray:
    y, _ = run(**inputs)
    return y


# revision 32
# speedup vs baseline: 1.5951x; 1.1616x over previous
"""MoE-routed group-norm kernel for Trainium2 (Bass/Tile), 8-core SPMD.

Problem (hardcoded shapes):
  x: [64, 512, 32, 32] f32
  experts_weight/bias: [8, 512], shared_weight/bias: [512]
  router_w: [8, 512], router_b: [8]

  flat = x.mean((2,3)); logits = flat @ router_w.T + router_b
  prob = softmax(logits); top-2 -> coeff = vals / sum(vals)
  fused_w = sum_k coeff_k * experts_weight[idx_k] + shared_weight (bias likewise)
  group-norm over G=32 groups of 16 channels, then y = x_norm * fused_w + fused_b

Strategy: data-parallel over batch, 8 samples per core. Channels on
partitions ([512,1024] = 4 chunks of [128,1024] per sample). The three
full-tensor streaming passes are spread across three engines so the kernel
stays DMA-bound:
  S1 (per-channel sums)    -> GpSimd reduce_sum
  S2 (per-channel sum x^2) -> ACT Square + accum_out (bulk out to scratch)
  pass2 (y = x*A + B)      -> DVE tensor_scalar (2x fp32 mode)
DMAs are HWDGE: loads on the SP ring (2 x 1 MiB per sample for early compute
start), stores on the ACT ring (2 MiB per sample). Bacc's finalize() splits
multi-sem waits (one-wait-per-instruction hardware constraint), which is what
makes HWDGE DMAs legal here.

All cross-partition steps (logits matvec, group-of-16 sums, group->channel
broadcast, expert mixing, [2,8]->[8,2] coeff transpose) are tiny PE matmuls
against constant masks, batched per PAIR of samples. Routing runs in a [2, E]
layout (pair on partitions, one Exp per pair): top-1 exp is exactly 1.0 and
the softmax denominator cancels in coeff = vals/sum(vals), so is_lt/is_ge
masking replaces any index math. rstd = 1/sqrt(var+eps) uses the bit-trick
seed + 3 Newton steps on DVE, keeping ACT's table pinned to exp_and_others
(exp/square/identity -> a single ACT table load for the whole kernel).
PSUM and ACT-written tiles use static per-pair regions (no slot reuse, no
cross-iteration WAW completion waits on PE/ACT).
"""

import numpy as np

import concourse.bacc as bacc
import concourse.bass as bass
import concourse.tile as tile
from concourse import mybir
from concourse.bass_utils import run_bass_kernel_spmd

F32 = mybir.dt.float32
I32 = mybir.dt.int32
ALU = mybir.AluOpType
ACTF = mybir.ActivationFunctionType
AXX = mybir.AxisListType.X

P = 128            # SBUF partitions
B, C, HWD = 64, 512, 1024
E, G = 8, 32
EPS = 1e-5
NCORES = 8
BPC = B // NCORES  # samples per core
NCH = C // P       # 4 channel chunks per sample
CPG = C // G       # 16 channels per group
PAIR = 2
RSQRT_MAGIC = 0x5F3759DF

# cA layout [128, 56]:
#   0:32  routerT   (routerT[p, 8j+e] = router_w[e, 128j+p] / 1024)
#   32:40 gmask     (1 if p//16 == g)
#   40:48 sw2       ((j, s): shared_weight[128j+p], replicated over s=0,1)
#   48:56 sb2      (same for shared_bias)
CA_W = 56
# cB layout [8, 1162]:
#   0:128 bmask | 128:640 ew | 640:1152 eb | 1152:1160 rb2 | 1160:1162 ident2
CB_W = 1162


def build(n_b: int = BPC) -> bass.Bass:
    assert n_b % PAIR == 0
    npair = n_b // PAIR
    nc = bacc.Bacc()
    x_d = nc.declare_dram_parameter("x", [n_b, C, HWD], F32, isOutput=False)
    ca_d = nc.declare_dram_parameter("ca", [P, CA_W], F32, isOutput=False)
    cb_d = nc.declare_dram_parameter("cb", [E, CB_W], F32, isOutput=False)
    y_d = nc.declare_dram_parameter("y", [n_b, C, HWD], F32, isOutput=True)

    with tile.TileContext(nc) as tc:
        with (
            tc.tile_pool(name="consts", bufs=1) as consts,
            tc.tile_pool(name="xp", bufs=6) as xp,
            tc.tile_pool(name="yp", bufs=3) as yp,
            tc.tile_pool(name="scr", bufs=6) as scrp,
            tc.tile_pool(name="statp", bufs=3) as statp,
            tc.tile_pool(name="tinyp", bufs=3) as tinyp,
            tc.tile_pool(name="ps_static", bufs=1, space="PSUM") as pstat,
        ):
            # consts staged through a DVE copy so PE inputs have DVE provenance
            ca_st = consts.tile([P, CA_W], F32)
            nc.sync.dma_start(out=ca_st, in_=ca_d[:, :])
            cb_st = consts.tile([E, CB_W], F32)
            nc.sync.dma_start(out=cb_st, in_=cb_d[:, :])
            ca = consts.tile([P, CA_W], F32)
            nc.vector.tensor_copy(ca, ca_st)
            cb = consts.tile([E, CB_W], F32)
            nc.vector.tensor_copy(cb, cb_st)
            zeros128 = consts.tile([P, 1], F32)
            nc.vector.memset(zeros128, 0.0)
            magic8 = consts.tile([E, PAIR * NCH], F32)
            nc.vector.memset(magic8[:, :].bitcast(I32), RSQRT_MAGIC)
            one8i = consts.tile([E, PAIR * NCH], F32)
            nc.vector.memset(one8i[:, :].bitcast(I32), 1)

            gmask = ca[:, 32:40]
            sw2 = ca[:, 40:48].rearrange("p (j s) -> p j s", s=PAIR)
            sb2 = ca[:, 48:56].rearrange("p (j s) -> p j s", s=PAIR)
            bmask = cb[:, 0:P]
            rb2 = cb[0:PAIR, 1152:1160]
            ident2 = cb[0:PAIR, 1160:1162]

            # static per-pair PSUM regions (never reused -> no PSUM WAW deps)
            ps_sm = pstat.tile([E, 26 * npair], F32, tag="sm")
            ps_fu = pstat.tile([P, npair, 8, PAIR], F32, tag="fu")
            ps_bc = pstat.tile([P, npair, PAIR, NCH, 2], F32, tag="bc")
            erow_all = consts.tile([PAIR, npair, E], F32)

            for ip in range(npair):
                xts = []
                s1 = statp.tile([P, PAIR, NCH], F32, tag="s1")
                s2 = statp.tile([P, PAIR, NCH], F32, tag="s2")
                for bb in range(PAIR):
                    b = ip * PAIR + bb
                    x_t = xp.tile([P, NCH, HWD], F32, tag="x")
                    xts.append(x_t)
                    xv = x_d[b].rearrange("(t p) f -> p t f", p=P)
                    # two 1 MiB loads so stats start after half the sample
                    nc.sync.dma_start(out=x_t[:, 0:2, :], in_=xv[:, 0:2, :])
                    nc.sync.dma_start(out=x_t[:, 2:4, :], in_=xv[:, 2:4, :])
                    for j in range(NCH):
                        nc.vector.reduce_sum(
                            s1[:, bb, j : j + 1], x_t[:, j, :], axis=AXX
                        )
                        sq = scrp.tile([P, HWD], F32, tag="sq")
                        nc.scalar.activation(
                            sq,
                            x_t[:, j, :],
                            ACTF.Square,
                            bias=zeros128,
                            scale=1.0,
                            accum_out=s2[:, bb, j : j + 1],
                        )

                o = 26 * ip
                gs1_ps = ps_sm[:, o : o + 8]          # group sums of S1 (bb,j)
                gs2_ps = ps_sm[:, o + 8 : o + 16]     # group sums of S2 (bb,j)
                lg_ps = ps_sm[0:PAIR, o + 16 : o + 24]  # logits [2, 8]
                ct_ps = ps_sm[:, o + 24 : o + 26]     # coeff^T [8, 2]

                # logits[s, e] = sum_c S1[c, s]/1024 * router_w[e, c]
                for j in range(NCH):
                    nc.tensor.matmul(
                        lg_ps,
                        s1[:, :, j],
                        ca[:, j * 8 : (j + 1) * 8],
                        start=(j == 0),
                        stop=(j == NCH - 1),
                    )
                nc.tensor.matmul(gs1_ps, gmask, s1[:, :, :])
                nc.tensor.matmul(gs2_ps, gmask, s2[:, :, :])

                # routing, pair-batched in [2, E] partition layout
                lrow = tinyp.tile([PAIR, E], F32, tag="lrow")
                nc.vector.tensor_tensor(lrow, lg_ps, rb2, ALU.add)
                nmax = tinyp.tile([PAIR, 1], F32, tag="nmax")
                nc.vector.reduce_max(nmax, lrow, axis=AXX, negate=True)
                erow = erow_all[:, ip, :]
                nc.scalar.activation(erow, lrow, ACTF.Exp, bias=nmax, scale=1.0)
                qrow = tinyp.tile([PAIR, E], F32, tag="qrow")
                nc.vector.scalar_tensor_tensor(
                    qrow, erow, 1.0, erow, op0=ALU.is_lt, op1=ALU.mult
                )
                m2 = tinyp.tile([PAIR, 1], F32, tag="m2")
                nc.vector.reduce_max(m2, qrow, axis=AXX)
                gate = tinyp.tile([PAIR, E], F32, tag="gate")
                nc.vector.scalar_tensor_tensor(
                    gate, erow, m2[:, 0:1], erow, op0=ALU.is_ge, op1=ALU.mult
                )
                den = tinyp.tile([PAIR, 1], F32, tag="den")
                nc.vector.tensor_scalar_add(den, m2, 1.0)
                rden = tinyp.tile([PAIR, 1], F32, tag="rden")
                nc.vector.reciprocal(rden, den)
                crow = tinyp.tile([PAIR, E], F32, tag="crow")
                nc.vector.tensor_scalar_mul(crow, gate, rden[:, 0:1])
                nc.tensor.matmul(ct_ps, crow, ident2)
                cT = tinyp.tile([E, PAIR], F32, tag="cT")
                nc.vector.tensor_copy(cT, ct_ps)

                # group stats -> mean, rstd in mr [8, (bb, j), 2]
                mr = statp.tile([E, PAIR, NCH, 2], F32, tag="mr")
                mean8 = mr[:, :, :, 0].rearrange("g b j -> g (b j)")
                nc.vector.tensor_scalar_mul(mean8, gs1_ps, 1.0 / (CPG * HWD))
                ex2 = tinyp.tile([E, PAIR * NCH], F32, tag="ex2")
                nc.vector.tensor_scalar_mul(ex2, gs2_ps, 1.0 / (CPG * HWD))
                mg2 = tinyp.tile([E, PAIR * NCH], F32, tag="mg2")
                nc.vector.tensor_tensor(mg2, mean8, mean8, ALU.mult)
                v = tinyp.tile([E, PAIR * NCH], F32, tag="v")
                nc.vector.scalar_tensor_tensor(
                    v, ex2, EPS, mg2, op0=ALU.add, op1=ALU.subtract
                )
                # rstd = rsqrt(v): bit-trick seed + 2 Newton iterations (DVE)
                yr = tinyp.tile([E, PAIR * NCH], F32, tag="yr")
                nc.vector.tensor_tensor(
                    yr[:, :].bitcast(I32),
                    v[:, :].bitcast(I32),
                    one8i[:, :].bitcast(I32),
                    ALU.arith_shift_right,
                )
                nc.vector.tensor_tensor(
                    yr[:, :].bitcast(I32),
                    magic8[:, :].bitcast(I32),
                    yr[:, :].bitcast(I32),
                    ALU.subtract,
                )
                t_a = tinyp.tile([E, PAIR * NCH], F32, tag="t_a")
                t_b = tinyp.tile([E, PAIR * NCH], F32, tag="t_b")
                for _ in range(2):
                    nc.vector.tensor_tensor(t_a, yr, yr, ALU.mult)
                    nc.vector.tensor_tensor(t_b, t_a, v, ALU.mult)
                    nc.vector.tensor_scalar(
                        t_a, t_b, -0.5, 1.5, op0=ALU.mult, op1=ALU.add
                    )
                    nc.vector.tensor_tensor(yr, yr, t_a, ALU.mult)
                nc.vector.tensor_copy(
                    mr[:, :, :, 1].rearrange("g b j -> g (b j)"), yr
                )

                # broadcast group stats to channels and mix expert tables
                bc = ps_bc[:, ip, :, :, :]
                nc.tensor.matmul(bc, bmask, mr[:, :, :, :])
                fu = ps_fu[:, ip, :, :]
                for j in range(NCH):
                    nc.tensor.matmul(
                        fu[:, j, :], cb[:, P + j * P : P + (j + 1) * P], cT
                    )
                    nc.tensor.matmul(
                        fu[:, NCH + j, :], cb[:, 640 + j * P : 640 + (j + 1) * P], cT
                    )

                # A = (fused_w + shared_w) * rstd ; B = (fused_b+shared_b) - mean*A
                bc_mean = bc[:, :, :, 0].rearrange("p b j -> p j b")
                bc_rstd = bc[:, :, :, 1].rearrange("p b j -> p j b")
                t1 = tinyp.tile([P, NCH, PAIR], F32, tag="t1")
                nc.vector.tensor_tensor(t1, fu[:, 0:NCH, :], sw2, ALU.add)
                At = tinyp.tile([P, NCH, PAIR], F32, tag="At")
                nc.vector.tensor_tensor(At, t1, bc_rstd, ALU.mult)
                t2 = tinyp.tile([P, NCH, PAIR], F32, tag="t2")
                nc.vector.tensor_tensor(t2, fu[:, NCH : 2 * NCH, :], sb2, ALU.add)
                t3 = tinyp.tile([P, NCH, PAIR], F32, tag="t3")
                nc.vector.tensor_tensor(t3, bc_mean, At, ALU.mult)
                Bt = tinyp.tile([P, NCH, PAIR], F32, tag="Bt")
                nc.vector.tensor_tensor(Bt, t2, t3, ALU.subtract)

                # pass2 on GpSimd (frees DVE/ACT), per-sample store on ACT ring
                for bb in range(PAIR):
                    b = ip * PAIR + bb
                    y_t = yp.tile([P, NCH, HWD], F32, tag="y")
                    for j in range(NCH):
                        if j < 2:
                            nc.vector.tensor_scalar(
                                y_t[:, j, :],
                                xts[bb][:, j, :],
                                At[:, j, bb : bb + 1],
                                Bt[:, j, bb : bb + 1],
                                op0=ALU.mult,
                                op1=ALU.add,
                            )
                        else:
                            nc.scalar.activation(
                                y_t[:, j, :],
                                xts[bb][:, j, :],
                                ACTF.Identity,
                                bias=Bt[:, j, bb : bb + 1],
                                scale=At[:, j, bb : bb + 1],
                            )
                    nc.gpsimd.dma_start(
                        out=y_d[b].rearrange("(t p) f -> p t f", p=P), in_=y_t
                    )
    nc.finalize()
    return nc


def pack_consts(
    experts_weight, experts_bias, shared_weight, shared_bias, router_w, router_b
):
    ca = np.zeros((P, CA_W), np.float32)
    ca[:, 0:32] = (
        (np.ascontiguousarray(router_w.T) / HWD)
        .reshape(NCH, P, E)
        .transpose(1, 0, 2)
        .reshape(P, 32)
    )
    pidx = np.arange(P)
    ca[:, 32:40] = (pidx[:, None] // CPG == np.arange(8)[None, :]).astype(np.float32)
    sw = shared_weight.reshape(NCH, P).T
    sb = shared_bias.reshape(NCH, P).T
    ca[:, 40:48] = np.repeat(sw, PAIR, axis=1)
    ca[:, 48:56] = np.repeat(sb, PAIR, axis=1)
    cb = np.zeros((E, CB_W), np.float32)
    cb[:, 0:P] = (np.arange(E)[:, None] == pidx[None, :] // CPG).astype(np.float32)
    cb[:, P : P + C] = experts_weight
    cb[:, P + C : P + 2 * C] = experts_bias
    cb[0:PAIR, 1152:1160] = router_b[None, :]
    cb[0:PAIR, 1160:1162] = np.eye(PAIR, dtype=np.float32)
    return ca, cb


_NC_CACHE: dict[int, bass.Bass] = {}


def _get_nc(n_b: int) -> bass.Bass:
    if n_b not in _NC_CACHE:
        _NC_CACHE[n_b] = build(n_b)
    return _NC_CACHE[n_b]


def run(
    x,
    experts_weight,
    experts_bias,
    shared_weight,
    shared_bias,
    router_w,
    router_b,
    trace: bool = False,
    tmpdir=None,
):
    x = np.ascontiguousarray(np.asarray(x, np.float32)).reshape(B, C, HWD)
    ca, cb = pack_consts(
        np.asarray(experts_weight, np.float32),
        np.asarray(experts_bias, np.float32),
        np.asarray(shared_weight, np.float32),
        np.asarray(shared_bias, np.float32),
        np.asarray(router_w, np.float32),
        np.asarray(router_b, np.float32),
    )
    nc = _get_nc(BPC)
    in_maps = [
        {"x": x[i * BPC : (i + 1) * BPC], "ca": ca, "cb": cb} for i in range(NCORES)
    ]
    res = run_bass_kernel_spmd(
        nc, in_maps, list(range(NCORES)), trace=trace, tmpdir=tmpdir
    )
    y = np.concatenate([res.results[i]["y"] for i in range(NCORES)], axis=0)
    return y.reshape(B, C, 32, 32), res


def kernel(**inputs) -> np.ndarray:
    y, _ = run(**inputs)
    return y
